# revision 5
# baseline (speedup 1.0000x reference)
"""Trainium2 Bass kernel v4: fused single-pass compressible-NS RHS.

Like v3 (fused chunks, bf16, folded tail, PE divergence) plus:
- ln/exp hoisted per chunk (2 activation-table loads per chunk, not 4)
- input z-derivative on PE (+/-identity matmuls) drained by ACT
- e-chain multiplies batched into 3-field ops; e_x mul on GpSimd
- slimmed carry copies / tail hop DMAs
- all weights packed into one DMA
"""

import sys

sys.path.insert(0, "/opt/trn_rl_repo")

import numpy as np

N = 192
NCORES = 8
CZ = 4  # flux planes computed per chunk

MU_REF = 1.8e-5
T_REF = 300.0
PR = 0.72
CP = 1005.0
C1 = N / 2.0
CPR = float(np.float32(CP / PR))
# mu' = K*sqrt(q(T)), q(T) = QA2*T^2 + QB*T + QG ~ T^1.4 on [0.5, 1.5]
QA = float(np.sqrt(0.30153127))
QB = 0.7893985
QG = -0.09243873
K2 = float((np.float32(MU_REF) * C1 * C1) ** 2)

# packed weight layout: columns [wd00 128][wd10x2 128][wbd 128][wi 128]
#                               [wn 128][wd01 64]
WCOLS = 128 * 5 + 64


def build_program(nz=24, num_devices=NCORES):
    import concourse.bacc as bacc
    import concourse.mybir as mybir
    from concourse.tile import TileContext

    assert nz % CZ == 0
    nch = nz // CZ

    bf = mybir.dt.bfloat16
    f32 = mybir.dt.float32
    Act = mybir.ActivationFunctionType
    nc = bacc.Bacc("TRN2", target_bir_lowering=False, debug=False,
                   num_devices=num_devices)

    in_d = nc.dram_tensor("in0", [nz + 4, N, 4, N], bf, kind="ExternalInput")
    wts_d = nc.dram_tensor("wts", [128, WCOLS], bf, kind="ExternalInput")
    out_d = nc.dram_tensor("out", [nz, N, 4, N], bf, kind="ExternalOutput")

    inv = in_d.ap()
    outv = out_d.ap()

    with TileContext(nc) as tc:
        with (
            tc.tile_pool(name="wpool", bufs=1) as wpool,
            tc.tile_pool(name="per", bufs=1) as per,
            tc.tile_pool(name="iop", bufs=2) as iop,
            tc.tile_pool(name="scr", bufs=1) as scr,
            tc.tile_pool(name="pin", bufs=2, space="PSUM") as pin,
            tc.tile_pool(name="pem", bufs=2, space="PSUM") as pem,
        ):
            wt = wpool.tile([128, WCOLS], bf, tag="wts")
            nc.sync.dma_start(out=wt[:, :], in_=wts_d.ap()[:, :])
            wd00 = wt[:, 0:128]
            wd10 = wt[:, 128:256]      # stacked base-0 / base-64 copies
            wbd = wt[:, 256:384]
            wi = wt[:, 384:512]
            wn = wt[:, 512:640]
            wd01 = wt[:, 640:704]

            # persistent flux tiles
            FM_t = per.tile([128, 3 * 4 * 6 * N], bf, tag="FM")
            FM = FM_t.rearrange("p (d f s x) -> p d f s x", d=3, f=4, s=6)
            TE_t = per.tile([128, 3 * 4 * 4 * N], bf, tag="TE")
            TE = TE_t.rearrange("p (d f s x) -> p d f s x", d=3, f=4, s=4)
            FMg = FM_t.rearrange("p (g s x) -> p g s x", g=12, s=6)
            TEg = TE_t.rearrange("p (g s x) -> p g s x", g=12, s=4)

            # ---------- helpers ----------
            def mk(pool, tag, dims):
                free = 1
                for d in dims:
                    free *= d
                t = pool.tile([128, free], bf, tag=tag)
                pat = " ".join(chr(97 + i) for i in range(len(dims)))
                return t.rearrange(
                    f"p ({pat}) -> p {pat}",
                    **{chr(97 + i): dims[i] for i in range(len(dims))})

            def indz_main(v0, GD, zslots, v1, v1slots, v1half):
                """in-plane y-deriv + z-deriv for main rows; one fused
                drain per (plane, fpair) into GD[:, i, (g1|dz), f, x]."""
                for i, zp in enumerate(zslots):
                    h = v1half[i]
                    w10 = wd10[0:64, 0:128] if h == 0 else wd10[64:128, 0:128]
                    v1s = v1[0:64] if h == 0 else v1[64:128]
                    for fp in range(2):
                        ps = pin.tile([128, 1024], f32, tag="pind")
                        psv = ps.rearrange("p (b k) -> p b k", b=2)
                        fsl = slice(2 * fp, 2 * fp + 2)
                        ra = psv[:, 0, 0:384].rearrange("p (f x) -> p f x",
                                                        f=2)
                        rb = psv[:, 1, 0:384].rearrange("p (f x) -> p f x",
                                                        f=2)
                        nc.tensor.matmul(ra[:, :, :], wd00,
                                         v0[:, zp, fsl, :],
                                         start=True, stop=False,
                                         skip_group_check=True)
                        nc.tensor.matmul(ra[:, :, :], w10,
                                         v1s[:, v1slots[i], fsl, :],
                                         start=False, stop=True,
                                         skip_group_check=True)
                        nc.tensor.matmul(rb[:, :, :], wi,
                                         v0[:, zp + 1, fsl, :],
                                         start=True, stop=False,
                                         skip_group_check=True)
                        nc.tensor.matmul(rb[:, :, :], wn,
                                         v0[:, zp - 1, fsl, :],
                                         start=False, stop=True,
                                         skip_group_check=True)
                        dst = GD[:, i, :, fsl, :]
                        srcv = psv[:, :, 0:384].rearrange(
                            "p b (f x) -> p b f x", f=2)
                        nc.scalar.copy(dst[:, :, :, :], srcv[:, :, :, :])

            def indz_tail(v0, GD, pair_specs):
                for (gs, z0, z1, v1t, v1s) in pair_specs:
                    for fp in range(2):
                        ps = pin.tile([128, 1024], f32, tag="pind")
                        psv = ps.rearrange("p (b k) -> p b k", b=2)
                        fsl = slice(2 * fp, 2 * fp + 2)
                        ra = psv[:, 0, 0:384].rearrange("p (f x) -> p f x",
                                                        f=2)
                        rb = psv[:, 1, 0:384].rearrange("p (f x) -> p f x",
                                                        f=2)
                        nc.tensor.matmul(ra[0:64, :, :], wd01,
                                         v0[:, z0, fsl, :],
                                         start=True, stop=True,
                                         skip_group_check=True)
                        nc.tensor.matmul(ra[64:128, :, :], wd01,
                                         v0[:, z1, fsl, :],
                                         start=True, stop=True,
                                         skip_group_check=True)
                        nc.tensor.matmul(ra[:, :, :], wbd,
                                         v1t[:, v1s, fsl, :],
                                         start=False, stop=True,
                                         skip_group_check=True)
                        nc.tensor.matmul(rb[:, :, :], wi,
                                         v1t[:, v1s + 1, fsl, :],
                                         start=True, stop=False,
                                         skip_group_check=True)
                        nc.tensor.matmul(rb[:, :, :], wn,
                                         v1t[:, v1s - 1, fsl, :],
                                         start=False, stop=True,
                                         skip_group_check=True)
                        dst = GD[:, gs, :, fsl, :]
                        srcv = psv[:, :, 0:384].rearrange(
                            "p b (f x) -> p b f x", f=2)
                        nc.scalar.copy(dst[:, :, :, :], srcv[:, :, :, :])

            def mu_block(muM, muT, TM, TT_, sM, sT):
                """mu (and CPR*mu) for main+tail via Square/Sqrt only."""
                nc.scalar.activation(sM, TM, Act.Square, scale=QA)
                nc.scalar.activation(sT, TT_, Act.Square, scale=QA)
                for (mv, s_, Tv) in ((muM, sM, TM), (muT, sT, TT_)):
                    nc.vector.tensor_scalar(mv[:, 1], Tv, QB, QG,
                                            mybir.AluOpType.mult,
                                            mybir.AluOpType.add)
                    nc.vector.tensor_add(mv[:, 1], mv[:, 1], s_)
                    nc.scalar.activation(mv[:, 0], mv[:, 1], Act.Sqrt,
                                         scale=K2)
                    nc.vector.tensor_scalar_mul(mv[:, 1], mv[:, 0], CPR)
                    nc.vector.tensor_scalar_mul(mv[:, 2], mv[:, 0], 2.0)

            def tau_stage(v, g1, dz, zc, mus, Fz, Fy, Fx, F3, Fz12v,
                          ub, psl=None):
                """Flux for zc planes. mus: [P, 2, zc, N] (mu, mut).
                F3: d -> 3-field flux block; Fy2: Fy fields 1:3 block."""
                ctr = v[:, 1:1 + zc, :, :]
                mu, mut = mus[:, 0], mus[:, 1]

                dxv = mk(scr, "dx", (4, 4, N))
                dxv = dxv[psl] if psl else dxv
                dx = dxv[:, 0:zc]
                nc.vector.tensor_sub(dx[:, :, :, 1:191],
                                     ctr[:, :, :, 2:192], ctr[:, :, :, 0:190])
                nc.vector.tensor_sub(dx[:, :, :, 0:192:191],
                                     ctr[:, :, :, 1::-1],
                                     ctr[:, :, :, 191:189:-1])

                qv = mk(scr, "q", (2, 4, N))
                qv = qv[psl] if psl else qv
                qv = qv[:, :, 0:zc]
                dv, qh = qv[:, 0], qv[:, 1]
                nc.gpsimd.tensor_add(dv[:, :, :], dz[:, :, 0, :],
                                     g1[:, :, 1, :])
                nc.gpsimd.tensor_add(dv[:, :, :], dv[:, :, :],
                                     dx[:, :, 2, :])
                nc.vector.tensor_scalar_mul(qh[:, :, :], dv[:, :, :],
                                            float(1.0 / 3.0))

                # off-diagonal sums (DVE) and g-qh diagonals (Pool),
                # written pre-mu straight into the flux slots
                mu2 = mus[:, 2]
                nc.vector.tensor_add(Fz[1], g1[:, :, 0, :], dz[:, :, 1, :])
                nc.vector.tensor_add(Fz[2], dx[:, :, 0, :], dz[:, :, 2, :])
                nc.vector.tensor_add(Fy[2], dx[:, :, 1, :], g1[:, :, 2, :])
                nc.gpsimd.tensor_sub(Fz[0], dz[:, :, 0, :], qh[:, :, :])
                nc.gpsimd.tensor_sub(Fy[1], g1[:, :, 1, :], qh[:, :, :])
                nc.gpsimd.tensor_sub(Fx[2], dx[:, :, 2, :], qh[:, :, :])

                # scale by mu (mu2 on diagonals) in place
                P = 64 if psl else 128
                mu_b2 = mus[:, 0:1].broadcast_to((P, 2, zc, N))
                nc.vector.tensor_mul(Fz[0], Fz[0], mu2)
                nc.vector.tensor_mul(Fz12v, Fz12v, mu_b2)
                nc.vector.tensor_mul(Fy[1], Fy[1], mu2)
                nc.vector.tensor_mul(Fy[2], Fy[2], mu)
                nc.vector.tensor_mul(Fx[2], Fx[2], mu2)
                nc.vector.tensor_copy(Fy[0], Fz[1])
                nc.vector.tensor_copy(Fx[0], Fz[2])
                nc.vector.tensor_copy(Fx[1], Fy[2])

                # energy fluxes: e_j = sum_i tau_ij u_i + mut * DjT
                pr = mk(scr, "pr", (3, 4, N))
                pr = pr[psl] if psl else pr
                pr = pr[:, :, 0:zc]
                t0v = mk(scr, "t0", (2, 4, N))
                t0v = t0v[psl] if psl else t0v
                t0v = t0v[:, :, 0:zc]
                t0, t1 = t0v[:, 0], t0v[:, 1]

                # e_z on DVE
                nc.vector.tensor_mul(pr[:, :, :, :], F3[0], ub)
                nc.vector.tensor_add(t0, pr[:, 0], pr[:, 1])
                nc.vector.tensor_mul(t1, mut, dz[:, :, 3, :])
                nc.vector.tensor_add(t0, t0, pr[:, 2])
                nc.vector.tensor_add(Fz[3], t0, t1)
                # e_x: batched mul on GpSimd, rest on DVE
                prx = mk(scr, "prx", (3, 4, N))
                prx = prx[psl] if psl else prx
                prx = prx[:, :, 0:zc]
                nc.gpsimd.tensor_mul(prx[:, :, :, :], F3[2], ub)
                nc.vector.tensor_add(t0, prx[:, 0], prx[:, 1])
                nc.vector.tensor_mul(t1, mut, dx[:, :, 3, :])
                nc.vector.tensor_add(t0, t0, prx[:, 2])
                nc.vector.tensor_add(Fx[3], t0, t1)
                # e_y fully on GpSimd
                pry = mk(scr, "pry", (3, 4, N))
                pry = pry[psl] if psl else pry
                pry = pry[:, :, 0:zc]
                g = nc.gpsimd
                g.tensor_mul(pry[:, :, :, :], F3[1], ub)
                g.tensor_add(pry[:, 0], pry[:, 0], pry[:, 1])
                g.tensor_mul(pry[:, 1], mut, g1[:, :, 3, :])
                g.tensor_add(pry[:, 0], pry[:, 0], pry[:, 2])
                g.tensor_add(Fy[3], pry[:, 0], pry[:, 1])

            def indy_main(v0, g1M, zslots, v1, v1slots, v1half):
                for i, zp in enumerate(zslots):
                    h = v1half[i]
                    w10 = wd10[0:64, 0:128] if h == 0 else wd10[64:128, 0:128]
                    v1s = v1[0:64] if h == 0 else v1[64:128]
                    for fp in range(2):
                        ps = pin.tile([128, 512], f32, tag="pin")
                        reg = ps[:, 0:384].rearrange("p (f x) -> p f x", f=2)
                        nc.tensor.matmul(reg[:, :, :], wd00,
                                         v0[:, zp, 2 * fp:2 * fp + 2, :],
                                         start=True, stop=False)
                        nc.tensor.matmul(reg[:, :, :], w10,
                                         v1s[:, v1slots[i],
                                             2 * fp:2 * fp + 2, :],
                                         start=False, stop=True)
                        nc.scalar.copy(g1M[:, i, 2 * fp:2 * fp + 2, :],
                                       reg[:, :, :])

            def indy_tail(v0, g1T, pair_specs):
                for (gs, z0, z1, v1t, v1s) in pair_specs:
                    for fp in range(2):
                        ps = pin.tile([128, 512], f32, tag="pin")
                        reg = ps[:, 0:384].rearrange("p (f x) -> p f x", f=2)
                        nc.tensor.matmul(reg[0:64, :, :], wd01,
                                         v0[:, z0, 2 * fp:2 * fp + 2, :],
                                         start=True, stop=True)
                        nc.tensor.matmul(reg[64:128, :, :], wd01,
                                         v0[:, z1, 2 * fp:2 * fp + 2, :],
                                         start=True, stop=True,
                                         skip_group_check=True)
                        nc.tensor.matmul(reg[:, :, :], wbd,
                                         v1t[:, v1s, 2 * fp:2 * fp + 2, :],
                                         start=False, stop=True,
                                         skip_group_check=True)
                        nc.scalar.copy(g1T[:, gs, 2 * fp:2 * fp + 2, :],
                                       reg[:, :, :])

            # ================= PROLOGUE =================
            v0p = mk(iop, "v0", (6, 4, N))
            nc.sync.dma_start(
                out=v0p[:, 0:4].rearrange("p z f x -> p z (f x)"),
                in_=inv[0:4, 0:128, :, :]
                .rearrange("z p f x -> z p (f x)").transpose([1, 0, 2]))
            v1p = mk(iop, "v1", (4, 4, N))
            nc.sync.dma_start(
                out=v1p[64:128].rearrange("p z f x -> p z (f x)"),
                in_=inv[0:4, 128:192, :, :]
                .rearrange("z p f x -> z p (f x)").transpose([1, 0, 2]))

            g1P = mk(scr, "g1M", (CZ, 4, N))
            indy_main(v0p, g1P, [1, 2], v1p, [1, 2], [1, 1])
            g1PT = mk(scr, "g1T", (2, 4, N))
            for i, zp in enumerate([1, 2]):
                for fp in range(2):
                    ps = pin.tile([128, 512], f32, tag="pin")
                    reg = ps[:, 0:384].rearrange("p (f x) -> p f x", f=2)
                    nc.tensor.matmul(reg[64:128, :, :], wbd[64:128, 64:128],
                                     v1p[64:128, zp, 2 * fp:2 * fp + 2, :],
                                     start=True, stop=False,
                                     skip_group_check=True)
                    nc.tensor.matmul(reg[64:128, :, :], wd01,
                                     v0p[:, zp, 2 * fp:2 * fp + 2, :],
                                     start=False, stop=True,
                                     skip_group_check=True)
                    nc.scalar.copy(g1PT[64:128, i, 2 * fp:2 * fp + 2, :],
                                   reg[64:128, :, :])

            dzP = mk(scr, "dzM", (CZ, 4, N))
            dz_pe(v0p, dzP, [(1, 0), (2, 1)])
            dzPT = mk(scr, "dzT", (2, 4, N))
            dz_pe(v1p[64:128], dzPT[64:128], [(1, 0), (2, 1)],
                  psl=slice(64, 128), skip=True)

            muM_t = mk(scr, "muM", (3, CZ, N))
            muT_t = mk(scr, "muT", (3, CZ, N))
            lnM = mk(scr, "lnM", (CZ, N))
            lnT = mk(scr, "lnT", (CZ, N))
            mu_block(muM_t[:, :, 0:2], muT_t[64:128, :, 0:2],
                     v0p[:, 1:3, 3, :], v1p[64:128, 1:3, 3, :],
                     lnM[:, 0:2], lnT[64:128, 0:2],
                     cbT=clnt[64:128])

            psl = slice(64, 128)
            tau_stage(v0p, g1P[:, 0:2], dzP[:, 0:2], 2, muM_t[:, :, 0:2],
                      {i: FM[:, 0, i, 4:6, :] for i in range(4)},
                      {i: FM[:, 1, i, 4:6, :] for i in range(4)},
                      {i: FM[:, 2, i, 4:6, :] for i in range(4)},
                      {d: FM[:, d, 0:3, 4:6, :] for d in range(3)},
                      v0p[:, 1:3, 0:3, :].transpose([0, 2, 1, 3]))
            tau_stage(v1p[64:128], g1PT[64:128], dzPT[64:128], 2,
                      muT_t[64:128, :, 0:2],
                      {i: TE[64:128, 0, i, 2:4, :] for i in range(4)},
                      {i: TE[64:128, 1, i, 2:4, :] for i in range(4)},
                      {i: TE[64:128, 2, i, 2:4, :] for i in range(4)},
                      {d: TE[64:128, d, 0:3, 2:4, :] for d in range(3)},
                      v1p[64:128, 1:3, 0:3, :].transpose([0, 2, 1, 3]),
                      psl=psl)

            # ================= CHUNKS =================
            for k in range(nch):
                a = CZ * k
                i0 = a + 2

                # carry: Fz slots 4,5 -> 0,1 ; Fy/Fx slot 5 -> 1
                nc.vector.tensor_copy(FMg[:, 0:4, 0:2, :],
                                      FMg[:, 0:4, 4:6, :])
                nc.vector.tensor_copy(FMg[:, 4:12, 1:2, :],
                                      FMg[:, 4:12, 5:6, :])
                # hopA: Fz slots 2,3 @hi -> 0,1 @lo ; Fy/Fx slot3 @hi -> 1 @lo
                nc.sync.dma_start(out=TEg[0:64, 0:4, 0:2, :],
                                  in_=TEg[64:128, 0:4, 2:4, :])
                nc.sync.dma_start(out=TEg[0:64, 4:12, 1:2, :],
                                  in_=TEg[64:128, 4:12, 3:4, :])

                v0 = mk(iop, "v0", (6, 4, N))
                nc.sync.dma_start(
                    out=v0.rearrange("p z f x -> p z (f x)"),
                    in_=inv[i0:i0 + 6, 0:128, :, :]
                    .rearrange("z p f x -> z p (f x)").transpose([1, 0, 2]))
                v1 = mk(iop, "v1", (4, 4, N))
                nc.sync.dma_start(
                    out=v1[0:64].rearrange("p z f x -> p z (f x)"),
                    in_=inv[i0:i0 + 4, 128:192, :, :]
                    .rearrange("z p f x -> z p (f x)").transpose([1, 0, 2]))
                nc.sync.dma_start(
                    out=v1[64:128].rearrange("p z f x -> p z (f x)"),
                    in_=inv[i0 + 2:i0 + 6, 128:192, :, :]
                    .rearrange("z p f x -> z p (f x)").transpose([1, 0, 2]))

                g1M = mk(scr, "g1M", (CZ, 4, N))
                indy_main(v0, g1M, [1, 2, 3, 4], v1, [1, 2, 1, 2],
                          [0, 0, 1, 1])
                g1T = mk(scr, "g1T", (2, 4, N))
                indy_tail(v0, g1T, [(0, 1, 3, v1, 1), (1, 2, 4, v1, 2)])

                dzM = mk(scr, "dzM", (CZ, 4, N))
                dz_pe(v0, dzM, [(1, 0), (2, 1), (3, 2), (4, 3)])
                dzT = mk(scr, "dzT", (2, 4, N))
                dz_pe(v1, dzT, [(1, 0), (2, 1)])

                muM_t = mk(scr, "muM", (3, CZ, N))
                muT_t = mk(scr, "muT", (3, CZ, N))
                lnM = mk(scr, "lnM", (CZ, N))
                lnT = mk(scr, "lnT", (CZ, N))
                mu_block(muM_t, muT_t[:, :, 0:2],
                         v0[:, 1:5, 3, :], v1[:, 1:3, 3, :],
                         lnM, lnT[:, 0:2])

                tau_stage(v0, g1M, dzM, CZ, muM_t,
                          {i: FM[:, 0, i, 2:6, :] for i in range(4)},
                          {i: FM[:, 1, i, 2:6, :] for i in range(4)},
                          {i: FM[:, 2, i, 2:6, :] for i in range(4)},
                          {d: FM[:, d, 0:3, 2:6, :] for d in range(3)},
                          v0[:, 1:5, 0:3, :].transpose([0, 2, 1, 3]))
                tau_stage(v1, g1T, dzT, 2, muT_t[:, :, 0:2],
                          {i: TE[:, 0, i, 2:4, :] for i in range(4)},
                          {i: TE[:, 1, i, 2:4, :] for i in range(4)},
                          {i: TE[:, 2, i, 2:4, :] for i in range(4)},
                          {d: TE[:, d, 0:3, 2:4, :] for d in range(3)},
                          v1[:, 1:3, 0:3, :].transpose([0, 2, 1, 3]))

                # hopB: Fz slots 2,3 @lo -> 0,1 @hi ; Fy/Fx slot3 @lo -> 1 @hi
                nc.sync.dma_start(out=TEg[64:128, 0:4, 0:2, :],
                                  in_=TEg[0:64, 0:4, 2:4, :])
                nc.sync.dma_start(out=TEg[64:128, 4:12, 1:2, :],
                                  in_=TEg[0:64, 4:12, 3:4, :])

                # ---- emission main ----
                outM = mk(iop, "outM", (CZ, 4, N))
                for i in range(CZ):
                    s = i + 1
                    if i == 0:
                        teh, tes, w10 = (TEprev[64:128], 1,
                                         wd10[64:128, 0:128])
                    elif i < 3:
                        teh, tes, w10 = TE[0:64], i - 1, wd10[0:64, 0:128]
                    else:
                        teh, tes, w10 = TE[64:128], 0, wd10[64:128, 0:128]
                    ps = pem.tile([128, 1024], f32, tag="pem")
                    psv = ps.rearrange("p (b k) -> p b k", b=2)
                    for fp in range(2):
                        reg = psv[:, fp, 0:384].rearrange(
                            "p (f x) -> p f x", f=2)
                        f0 = 2 * fp
                        nc.tensor.matmul(reg[:, :, :], wi,
                                         FM[:, 0, f0:f0 + 2, s + 1, :],
                                         start=True, stop=False,
                                         skip_group_check=True)
                        nc.tensor.matmul(reg[:, :, :], wd00,
                                         FM[:, 1, f0:f0 + 2, s, :],
                                         start=False, stop=False,
                                         skip_group_check=True)
                        nc.tensor.matmul(reg[:, :, :], w10,
                                         teh[:, 1, f0:f0 + 2, tes, :],
                                         start=False, stop=False,
                                         skip_group_check=True)
                        for ff in range(2):
                            Fxc = FM[:, 2, f0 + ff, s, :]
                            nc.tensor.matmul(reg[:, ff, 0:191], wi,
                                             Fxc[:, 1:192],
                                             start=False, stop=False,
                                         skip_group_check=True)
                            nc.tensor.matmul(reg[:, ff, 191:192], wi,
                                             Fxc[:, 0:1],
                                             start=False, stop=False,
                                         skip_group_check=True)
                            nc.tensor.matmul(reg[:, ff, 1:192], wn,
                                             Fxc[:, 0:191],
                                             start=False, stop=False,
                                         skip_group_check=True)
                            nc.tensor.matmul(reg[:, ff, 0:1], wn,
                                             Fxc[:, 191:192],
                                             start=False, stop=False,
                                         skip_group_check=True)
                        nc.tensor.matmul(reg[:, :, :], wn,
                                         FM[:, 0, f0:f0 + 2, s - 1, :],
                                         start=False, stop=True,
                                         skip_group_check=True)
                    dst = outM[:, i, :, :].rearrange("p (b f) x -> p b f x",
                                                     b=2)
                    srcv = psv[:, :, 0:384].rearrange(
                        "p b (f x) -> p b f x", f=2)
                    nc.scalar.copy(dst[:, :, :, :], srcv[:, :, :, :])

                # ---- emission tail ----
                outT = mk(iop, "outT", (2, 4, N))

                def tloc(prel):
                    """tail flux source for plane a+prel: (tile, half, slot).
                    half 0 = parts 0:64, half 1 = parts 64:128."""
                    if prel <= 0:
                        return (TEprev, 1, prel + 1)
                    if prel <= 2:
                        return (TE, 0, prel - 1)
                    return (TE, 1, prel - 3)

                def wblk(w, sh, oh):
                    del oh
                    return w[64 * sh:64 * sh + 64, 64 * sh:64 * sh + 64]

                for j in range(2):
                    s = j + 1
                    ps = pem.tile([128, 1024], f32, tag="pem")
                    psv = ps.rearrange("p (b k) -> p b k", b=2)
                    for fp in range(2):
                        reg = psv[:, fp, 0:384].rearrange(
                            "p (f x) -> p f x", f=2)
                        f0 = 2 * fp
                        fsl = slice(f0, f0 + 2)
                        started = [False, False]

                        def acc(d, prel_lo, w128, w64, xsl=None, osl=None,
                                last=False):
                            """Accumulate flux dir d at planes
                            (a+prel_lo, a+prel_lo+2) into reg halves."""
                            la, lb = tloc(prel_lo), tloc(prel_lo + 2)
                            fused = (la[0] is lb[0] and la[2] == lb[2]
                                     and la[1] == 0 and lb[1] == 1)
                            xs = xsl if xsl is not None else slice(0, N)
                            os_ = osl if osl is not None else slice(0, N)
                            if fused:
                                mv = la[0][:, d, fsl, la[2], xs]
                                st = not (started[0] and started[1])
                                nc.tensor.matmul(reg[:, :, os_], w128,
                                                 mv, start=st,
                                                 stop=last,
                                                 skip_group_check=True)
                                started[0] = started[1] = True
                                return
                            for oh, (tt, sh, sl) in ((0, la), (1, lb)):
                                mv = tt[64 * sh:64 * sh + 64, d, fsl, sl, xs]
                                r = reg[64 * oh:64 * oh + 64, :, os_]
                                st = not started[oh]
                                nc.tensor.matmul(r, wblk(w64, sh, oh), mv,
                                                 start=st, stop=last,
                                                 skip_group_check=True)
                                started[oh] = True

                        # Fz(p+1), Fz(p-1), Fy(p) via Dy, Fx(p) shifts
                        acc(0, s, wi, wi)
                        acc(0, s - 2, wn, wn)
                        # Dy K0 from main rows
                        nc.tensor.matmul(reg[0:64, :, :], wd01,
                                         FM[:, 1, fsl, s, :],
                                         start=False, stop=False,
                                         skip_group_check=True)
                        nc.tensor.matmul(reg[64:128, :, :], wd01,
                                         FM[:, 1, fsl, s + 2, :],
                                         start=False, stop=False,
                                         skip_group_check=True)
                        # Dy K64 from tail rows
                        acc(1, s - 1, wbd, wbd)
                        # Fx shifts (per field)
                        for ff in range(2):
                            fsl_save = fsl
                            fsl = slice(f0 + ff, f0 + ff + 1)
                            acc(2, s - 1, wi, wi,
                                xsl=slice(1, 192), osl=slice(0, 191))
                            acc(2, s - 1, wi, wi,
                                xsl=slice(0, 1), osl=slice(191, 192))
                            acc(2, s - 1, wn, wn,
                                xsl=slice(0, 191), osl=slice(1, 192))
                            acc(2, s - 1, wn, wn,
                                xsl=slice(191, 192), osl=slice(0, 1),
                                last=(ff == 1))
                            fsl = fsl_save
                    dst = outT[:, j, :, :].rearrange("p (b f) x -> p b f x",
                                                     b=2)
                    srcv = psv[:, :, 0:384].rearrange(
                        "p b (f x) -> p b f x", f=2)
                    nc.scalar.copy(dst[:, :, :, :], srcv[:, :, :, :])

                nc.sync.dma_start(
                    out=outv[a:a + 4, 0:128, :, :]
                    .rearrange("z p f x -> z p (f x)").transpose([1, 0, 2]),
                    in_=outM.rearrange("p z f x -> p z (f x)"))
                nc.sync.dma_start(
                    out=outv[a:a + 2, 128:192, :, :]
                    .rearrange("z p f x -> z p (f x)").transpose([1, 0, 2]),
                    in_=outT[0:64].rearrange("p z f x -> p z (f x)"))
                nc.sync.dma_start(
                    out=outv[a + 2:a + 4, 128:192, :, :]
                    .rearrange("z p f x -> z p (f x)").transpose([1, 0, 2]),
                    in_=outT[64:128].rearrange("p z f x -> p z (f x)"))

    nc.compile()
    return nc


_NC_CACHE = None


def _get_nc():
    global _NC_CACHE
    if _NC_CACHE is None:
        _NC_CACHE = build_program()
    return _NC_CACHE


def make_weights():
    import ml_dtypes
    bf = ml_dtypes.bfloat16
    dm = np.zeros((N, N), dtype=np.float32)
    for m in range(N):
        dm[m, (m + 1) % N] = 1.0
        dm[m, (m - 1) % N] = -1.0
    dyt = np.ascontiguousarray(dm.T)
    B = dyt[128:192, 128:192]
    wts = np.zeros((128, WCOLS), dtype=np.float32)
    wts[:, 0:128] = dyt[0:128, 0:128]                    # wd00
    wts[0:64, 128:256] = dyt[128:192, 0:128]             # wd10 base-0
    wts[64:128, 128:256] = dyt[128:192, 0:128]           # wd10 base-64
    wts[0:64, 256:320] = B                               # wbd blockdiag
    wts[64:128, 320:384] = B
    wts[:, 384:512] = np.eye(128)                        # wi
    wts[:, 512:640] = -np.eye(128)                       # wn
    wts[:, 640:704] = dyt[0:128, 128:192]                # wd01
    return {"wts": wts.astype(bf)}


def shard_inputs(u, T, nz=24, ncores=NCORES):
    import ml_dtypes
    bf = ml_dtypes.bfloat16
    w = make_weights()
    in_maps = []
    for k in range(ncores):
        idx = np.arange(nz * k - 2, nz * k + nz + 2) % N
        arr = np.empty((nz + 4, N, 4, N), dtype=bf)
        arr[:, :, 0:3, :] = u[:, idx, :, :].transpose(1, 2, 0, 3)
        arr[:, :, 3, :] = T[idx, :, :]
        m = {"in0": arr}
        m.update(w)
        in_maps.append(m)
    return in_maps


def kernel(u: np.ndarray, T: np.ndarray) -> np.ndarray:
    from concourse.bass_utils import run_bass_kernel_spmd

    u = np.asarray(u, dtype=np.float32)
    T = np.asarray(T, dtype=np.float32)
    nc = _get_nc()
    nz = N // NCORES
    in_maps = shard_inputs(u, T, nz=nz)
    res = run_bass_kernel_spmd(nc, in_maps, list(range(NCORES)))

    out = np.zeros((5, N, N, N), dtype=np.float32)
    for k in range(NCORES):
        o = np.asarray(res.results[k]["out"])
        out[1:5, nz * k:nz * k + nz, :, :] = o.transpose(2, 0, 1, 3) \
            .astype(np.float32)
    return out
